# revision 1
# baseline (speedup 1.0000x reference)
"""Trainium2 Bass kernel for nn_ContrastiveLoss3DTo2D.

Reference computation (B=256, D=1024, margin=0.2):
    scores[i, j] = dot(im[j], s[i, j])                    # [B, B]
    cost_s  = sum_i relu(margin + max_{j!=i} scores[i,j] - scores[i,i])
    cost_im = sum_j relu(margin + max_{i!=j} scores[i,j] - scores[j,j])
    loss = cost_s + cost_im

Sharding: s (and the score matrix) is sharded along i across 8 cores
(32 rows each); im is replicated. Each core computes its 32x256 score
block via a fused DVE multiply+reduce while streaming its 32 MB shard
from HBM, then reduces on-device to tiny per-core partials:
  rowcost[32]  - relu(margin + rowmax_offdiag - diag) per local row
  colmax[256]  - per-column max over local rows (diagonal masked out)
  diagv[32]    - local diagonal scores
The host combines partials (max over cores for columns, sums) into the
scalar loss. relu/max commute (relu is monotone), so per-core column
maxima compose exactly.
"""

import numpy as np

B = 256
D = 1024
M = 8            # cores
BL = B // M      # 32 local rows per core
P = 128          # SBUF partitions
T = B // P       # 2 column tiles of 128
MARGIN = 0.2
CHUNK = 4        # s rows per DMA (4 MB transfers)
NEG = -1.0e30    # diagonal mask value
NEG_INIT = -3.0e38

_NC = None


def _build_nc():
    import concourse.bacc as bacc
    from concourse import mybir
    from concourse.tile import TileContext

    f32 = mybir.dt.float32
    add = mybir.AluOpType.add
    mult = mybir.AluOpType.mult
    amax = mybir.AluOpType.max

    nc = bacc.Bacc(None, target_bir_lowering=False, debug=False)
    im_d = nc.declare_dram_parameter("im", [B, D], f32, isOutput=False)
    s_d = nc.declare_dram_parameter("s", [BL, B, D], f32, isOutput=False)
    mt_d = nc.declare_dram_parameter("mask_t_neg", [B, BL], f32, isOutput=False)
    nr_d = nc.declare_dram_parameter("neg_rows", [BL, B], f32, isOutput=False)
    er_d = nc.declare_dram_parameter("eye_rows", [BL, B], f32, isOutput=False)
    rc_d = nc.declare_dram_parameter("rowcost", [BL, 1], f32, isOutput=True)
    cm_d = nc.declare_dram_parameter("colmax", [B, 1], f32, isOutput=True)
    dg_d = nc.declare_dram_parameter("diagv", [BL, 1], f32, isOutput=True)

    with TileContext(nc) as tc:
        with (
            tc.tile_pool(name="const", bufs=1) as cpool,
            tc.tile_pool(name="sload", bufs=4) as spool,
            tc.tile_pool(name="scratch", bufs=3) as prpool,
            tc.tile_pool(name="small", bufs=1) as smpool,
        ):
            # im packed as [p, t*D + d] so j = t*128 + p matches the s tiles.
            # First on the Sync ring, ahead of the s stream.
            im_t = cpool.tile([P, T * D], f32, tag="im")
            nc.sync.dma_start(
                out=im_t[:].rearrange("p (t d) -> p t d", t=T),
                in_=im_d[:].rearrange("(t p) d -> p t d", p=P),
            )

            # Epilogue masks ride the otherwise-idle ACT HWDGE ring at t=0.
            mt_t = cpool.tile([P, T * BL], f32, tag="maskT")
            nc.scalar.dma_start(
                out=mt_t[:].rearrange("p (t i) -> p t i", t=T),
                in_=mt_d[:].rearrange("(t p) i -> p t i", p=P),
            )
            nr_t = cpool.tile([BL, B], f32, tag="negrows")
            nc.scalar.dma_start(out=nr_t[:], in_=nr_d[:])
            er_t = cpool.tile([BL, B], f32, tag="eyerows")
            nc.scalar.dma_start(out=er_t[:], in_=er_d[:])

            # scores^T: partition = column j (within tile t), free = local row i
            scoresT = smpool.tile([P, T * BL], f32, tag="scoresT")

            # Ramped chunk sizes: small first chunks so the first multiply
            # starts as soon as ~1 MB has landed, then steady 4-row chunks.
            chunk_rows = [1, 1, 2] + [CHUNK] * ((BL - 4) // CHUNK)
            assert sum(chunk_rows) == BL
            row0 = 0
            for nr in chunk_rows:
                s_t = spool.tile([P, nr * T * D], f32, tag="s")
                nc.sync.dma_start(
                    out=s_t[:, 0:nr * T * D].rearrange(
                        "p (r t d) -> p r t d", r=nr, t=T
                    ),
                    in_=s_d[row0:row0 + nr].rearrange(
                        "r (t p) d -> p r t d", p=P
                    ),
                )
                for r in range(nr):
                    i = row0 + r
                    off = r * T * D
                    prod = prpool.tile([P, T * D], f32, tag="prod")
                    # DVE: one fused product over both column halves.
                    nc.vector.tensor_mul(
                        prod[:], s_t[:, off:off + T * D], im_t[:]
                    )
                    # Free-axis sums: ACT accumulates most halves; DVE
                    # takes every 4th i's t=1 half to balance engine load.
                    nc.scalar.activation(
                        out=prod[:, 0:D], in_=prod[:, 0:D],
                        func=mybir.ActivationFunctionType.Copy,
                        accum_out=scoresT[:, i:i + 1],
                    )
                    if i % 4 == 3:
                        nc.vector.reduce_sum(
                            scoresT[:, BL + i:BL + i + 1], prod[:, D:2 * D],
                            axis=mybir.AxisListType.X,
                        )
                    else:
                        nc.scalar.activation(
                            out=prod[:, D:2 * D], in_=prod[:, D:2 * D],
                            func=mybir.ActivationFunctionType.Copy,
                            accum_out=scoresT[:, BL + i:BL + i + 1],
                        )
                row0 += nr

            # Column maxima over local rows, diagonal masked to -1e30.
            colmax = smpool.tile([P, T], f32, tag="colmax")
            for t in range(T):
                mscr = prpool.tile([P, BL], f32, tag="mscr")
                nc.vector.tensor_add(
                    mscr[:],
                    scoresT[:, t * BL:(t + 1) * BL],
                    mt_t[:, t * BL:(t + 1) * BL],
                )
                nc.vector.reduce_max(
                    colmax[:, t:t + 1], mscr[:], axis=mybir.AxisListType.X
                )

            # Transpose scores^T -> rows [32, 256] via 32x32 stream blocks.
            rows = smpool.tile([BL, B], f32, tag="rows")
            for t in range(T):
                for k in range(P // 32):
                    nc.vector.transpose(
                        out=rows[0:BL, t * P + k * 32:t * P + (k + 1) * 32],
                        in_=scoresT[k * 32:(k + 1) * 32, t * BL:(t + 1) * BL],
                    )

            # rowmax (diag masked), diag, then rowcost = relu(margin + rowmax - diag)
            rowstat = smpool.tile([BL, 4], f32, tag="rowstat")
            rs1 = prpool.tile([BL, B], f32, tag="rscr")
            nc.vector.tensor_add(rs1[:], rows[:], nr_t[:])
            nc.vector.reduce_max(
                rowstat[:, 0:1], rs1[:], axis=mybir.AxisListType.X
            )
            rs2 = prpool.tile([BL, B], f32, tag="rscr")
            nc.vector.tensor_mul(rs2[:], rows[:], er_t[:])
            nc.vector.reduce_sum(
                rowstat[:, 1:2], rs2[:], axis=mybir.AxisListType.X
            )
            nc.vector.tensor_sub(rowstat[:, 2:3], rowstat[:, 0:1], rowstat[:, 1:2])
            nc.vector.tensor_scalar(
                out=rowstat[:, 3:4], in0=rowstat[:, 2:3],
                scalar1=MARGIN, scalar2=0.0, op0=add, op1=amax,
            )

            nc.sync.dma_start(out=rc_d[:], in_=rowstat[:, 3:4])
            nc.sync.dma_start(out=dg_d[:], in_=rowstat[:, 1:2])
            for t in range(T):
                nc.sync.dma_start(
                    out=cm_d[t * P:(t + 1) * P], in_=colmax[:, t:t + 1]
                )

    nc.compile()
    return nc


def _get_nc():
    global _NC
    if _NC is None:
        _NC = _build_nc()
    return _NC


def _make_in_maps(im, s):
    il = np.arange(BL)
    in_maps = []
    for c in range(M):
        mt = np.zeros((B, BL), np.float32)
        nr = np.zeros((BL, B), np.float32)
        er = np.zeros((BL, B), np.float32)
        mt[c * BL + il, il] = NEG
        nr[il, c * BL + il] = NEG
        er[il, c * BL + il] = 1.0
        in_maps.append({
            "im": im,
            "s": s[c * BL:(c + 1) * BL],
            "mask_t_neg": mt,
            "neg_rows": nr,
            "eye_rows": er,
        })
    return in_maps


def _combine(results):
    rowcosts = np.concatenate([results[c]["rowcost"][:, 0] for c in range(M)])
    colmax = np.max(
        np.stack([results[c]["colmax"][:, 0] for c in range(M)]), axis=0
    )
    diag = np.concatenate([results[c]["diagv"][:, 0] for c in range(M)])
    cost_im = np.maximum(np.float32(MARGIN) + colmax - diag, np.float32(0.0))
    loss = rowcosts.sum(dtype=np.float32) + cost_im.sum(dtype=np.float32)
    return np.array(loss, dtype=np.float32)


def _run(im, s, **spmd_kwargs):
    from concourse.bass_utils import run_bass_kernel_spmd

    im = np.ascontiguousarray(np.asarray(im), dtype=np.float32)
    s = np.ascontiguousarray(np.asarray(s), dtype=np.float32)
    nc = _get_nc()
    res = run_bass_kernel_spmd(nc, _make_in_maps(im, s), list(range(M)),
                               **spmd_kwargs)
    return _combine(res.results), res


def kernel(im, s):
    loss, _ = _run(im, s)
    return loss



# revision 11
# speedup vs baseline: 1.3860x; 1.3860x over previous
"""Trainium2 Bass kernel for nn_ContrastiveLoss3DTo2D.

Reference computation (B=256, D=1024, margin=0.2):
    scores[i, j] = dot(im[j], s[i, j])                    # [B, B]
    cost_s  = sum_i relu(margin + max_{j!=i} scores[i,j] - scores[i,i])
    cost_im = sum_j relu(margin + max_{i!=j} scores[i,j] - scores[j,j])
    loss = cost_s + cost_im

Sharding: s (and the score matrix) is sharded along i across 8 cores
(32 rows each); im is replicated. Inputs are cast to fp16 on the host
(loss tolerance is 2e-2; fp16 keeps the dot-product error ~1e-4 rel)
which halves HBM traffic — the binding constraint — and doubles DVE
elementwise throughput. Each core streams its 16 MB shard and computes
the 32x256 score block with ONE fused DVE pass per (row, half):
tensor_tensor_reduce does multiply + free-axis accumulate (fp32) in a
single instruction, leaving ACT/PE idle.

Column layout: j = 2p + u (partition p in [0,128), u in {0,1}) so each
DMA descriptor is a contiguous 4 KB run (two adjacent j rows of D).
Per-core reductions produce colmax[256] (diag masked), diag[32], and
rowcost[32], packed via 32x32 stream transposes into a single [4,128]
fp32 tensor written with one 4-descriptor DMA (per-partition-column
outputs would emit hundreds of 4-byte descriptors that crawl for >10us).
The host combines per-core partials exactly as relu/max commute.
"""

import numpy as np

B = 256
D = 1024
M = 8            # cores
BL = B // M      # 32 local rows per core
P = 128          # SBUF partitions
U = 2            # j = 2p + u column interleave
MARGIN = 0.2
NEG = -1.0e30    # diagonal mask value
NEG_INIT = -3.0e38

_NC = None


def _build_nc():
    import concourse.bacc as bacc
    from concourse import mybir
    from concourse.tile import TileContext

    f32 = mybir.dt.float32
    f16 = mybir.dt.float16
    add = mybir.AluOpType.add
    mult = mybir.AluOpType.mult
    amax = mybir.AluOpType.max

    nc = bacc.Bacc(None, target_bir_lowering=False, debug=False)
    im_d = nc.declare_dram_parameter("im", [B, D], f16, isOutput=False)
    s_d = nc.declare_dram_parameter("s", [BL, B, D], f16, isOutput=False)
    mt_d = nc.declare_dram_parameter("mask_t_neg", [P, U * BL], f32, isOutput=False)
    nr_d = nc.declare_dram_parameter("neg_rows", [BL, B], f32, isOutput=False)
    er_d = nc.declare_dram_parameter("eye_rows", [BL, B], f32, isOutput=False)
    o_d = nc.declare_dram_parameter("out", [4, P], f32, isOutput=True)

    with TileContext(nc) as tc:
        with (
            tc.tile_pool(name="const", bufs=1) as cpool,
            tc.tile_pool(name="sload", bufs=4) as spool,
            tc.tile_pool(name="scratch", bufs=2) as prpool,
            tc.tile_pool(name="small", bufs=1) as smpool,
        ):
            # im packed as [p, u*D + d] with j = 2p + u, matching s tiles.
            # Rides the ACT HWDGE ring with the masks; s opens on Sync.
            im_t = cpool.tile([P, U * D], f16, tag="im")
            nc.scalar.dma_start(
                out=im_t[:].rearrange("p (u d) -> p u d", u=U),
                in_=im_d[:].rearrange("(p u) d -> p u d", p=P),
            )
            mt_t = cpool.tile([P, U * BL], f32, tag="maskT")
            nc.scalar.dma_start(out=mt_t[:], in_=mt_d[:])
            nr_t = cpool.tile([BL, B], f32, tag="negrows")
            nc.scalar.dma_start(out=nr_t[:], in_=nr_d[:])
            er_t = cpool.tile([BL, B], f32, tag="eyerows")
            nc.scalar.dma_start(out=er_t[:], in_=er_d[:])

            # scores^T: partition p, free column u*BL + i  (j = 2p + u)
            scoresT = smpool.tile([P, U * BL], f32, tag="scoresT")

            # Ramped chunks: tiny first so the first fused pass starts
            # ~1.5us in; tiny last so the stream tail is short.
            chunk_rows = [1, 1, 2, 4, 4, 4, 4, 4, 4, 2, 1, 1]
            assert sum(chunk_rows) == BL
            row0 = 0
            for ci, nr in enumerate(chunk_rows):
                s_t = spool.tile([P, nr * U * D], f16, tag="s")
                ring = nc.sync if ci % 2 == 0 else nc.scalar
                ring.dma_start(
                    out=s_t[:, 0:nr * U * D].rearrange(
                        "p (r f) -> p r f", r=nr
                    ),
                    in_=s_d[row0:row0 + nr].rearrange(
                        "r (p u) d -> p r (u d)", p=P
                    ),
                )
                for r in range(nr):
                    i = row0 + r
                    for u in range(U):
                        off = (r * U + u) * D
                        prod = prpool.tile([P, D], f16, tag="prod")
                        # Fused multiply + fp32 row-sum: one DVE pass.
                        # (tensor_tensor_reduce crashes TRN2 hardware;
                        # scalar_tensor_tensor's sum-accum is the safe
                        # fused path: out=(s*1)*im, accum=sum.)
                        nc.vector.scalar_tensor_tensor(
                            out=prod[:],
                            in0=s_t[:, off:off + D],
                            scalar=1.0,
                            in1=im_t[:, u * D:(u + 1) * D],
                            op0=mult,
                            op1=mult,
                            accum_out=scoresT[:, u * BL + i:u * BL + i + 1],
                        )
                row0 += nr

            # Packed output tile: col 0/1 = colmax (u=0/1), col 2 = diag,
            # col 3 = rowcost. Transposed at the end into [4, 128]. The
            # memset covers the pad lanes the transposes read.
            out_t = smpool.tile([P, 32], f32, tag="out_t")
            nc.gpsimd.memset(out_t[:], 0.0)

            # Column maxima over local rows, diagonal masked to -1e30:
            # fused (scoresT + mask) then max-reduce.
            for u in range(U):
                cscr = prpool.tile([P, BL], f32, tag="cscr")
                nc.vector.tensor_add(
                    cscr[:],
                    scoresT[:, u * BL:(u + 1) * BL],
                    mt_t[:, u * BL:(u + 1) * BL],
                )
                nc.vector.reduce_max(
                    out_t[:, u:u + 1], cscr[:], axis=mybir.AxisListType.X
                )

            # Transpose scores^T -> rows [32, 256] via 32x32 stream blocks.
            # rows[i, u*128 + pp] = score(i, j=2*pp+u).
            rows = smpool.tile([BL, B], f32, tag="rows")
            for u in range(U):
                for k in range(P // 32):
                    nc.vector.transpose(
                        out=rows[0:BL, u * P + k * 32:u * P + (k + 1) * 32],
                        in_=scoresT[k * 32:(k + 1) * 32, u * BL:(u + 1) * BL],
                    )

            # rowmax (diag masked) and diag, both fused single passes.
            rowstat = smpool.tile([BL, 4], f32, tag="rowstat")
            rscr1 = prpool.tile([BL, B], f32, tag="rscr")
            nc.vector.tensor_add(rscr1[:], rows[:], nr_t[:])
            nc.vector.reduce_max(
                rowstat[:, 0:1], rscr1[:], axis=mybir.AxisListType.X
            )
            rscr2 = prpool.tile([BL, B], f32, tag="rscr")
            # diag = sum(rows * eye) fused in one pass
            nc.vector.scalar_tensor_tensor(
                out=rscr2[:],
                in0=rows[:],
                scalar=1.0,
                in1=er_t[:],
                op0=mult,
                op1=mult,
                accum_out=out_t[0:BL, 2:3],
            )
            # rowcost = relu(margin + rowmax - diag)
            nc.vector.tensor_sub(rowstat[:, 1:2], rowstat[:, 0:1], out_t[0:BL, 2:3])
            nc.vector.tensor_scalar(
                out=out_t[0:BL, 3:4], in0=rowstat[:, 1:2],
                scalar1=MARGIN, scalar2=0.0, op0=add, op1=amax,
            )

            # Pack: transpose out_t's first 4 columns into rows of outT,
            # then ONE 4-descriptor DMA (512B per partition line).
            outT = smpool.tile([32, P], f32, tag="outT")
            for k in range(P // 32):
                nc.vector.transpose(
                    out=outT[0:32, k * 32:(k + 1) * 32],
                    in_=out_t[k * 32:(k + 1) * 32, 0:32],
                )
            nc.scalar.dma_start(out=o_d[:], in_=outT[0:4, 0:P])

    nc.compile()
    return nc


def _get_nc():
    global _NC
    if _NC is None:
        _NC = _build_nc()
    return _NC


def _make_in_maps(im, s):
    im16 = im.astype(np.float16)
    s16 = s.astype(np.float16)
    il = np.arange(BL)
    # column q in `rows` layout: q = u*128 + pp  <->  j = 2*pp + u
    jq = 2 * (np.arange(B) % P) + (np.arange(B) // P)
    in_maps = []
    for c in range(M):
        jdiag = c * BL + il                      # global row index of local i
        mt = np.zeros((P, U * BL), np.float32)   # mt[p, u*BL+i]
        pd, ud = jdiag % P, jdiag // P
        # j = 2p+u == jdiag  =>  p = jdiag//2, u = jdiag%2
        mt[jdiag // 2, (jdiag % 2) * BL + il] = NEG
        nr = np.zeros((BL, B), np.float32)
        er = np.zeros((BL, B), np.float32)
        qdiag = (jdiag % 2) * P + jdiag // 2     # q with j(q) == jdiag
        nr[il, qdiag] = NEG
        er[il, qdiag] = 1.0
        in_maps.append({
            "im": im16,
            "s": s16[c * BL:(c + 1) * BL],
            "mask_t_neg": mt,
            "neg_rows": nr,
            "eye_rows": er,
        })
    return in_maps


def _combine(results):
    colmax = np.full(B, -np.inf, np.float32)
    rowcosts = np.empty(B, np.float32)
    diag = np.empty(B, np.float32)
    for c in range(M):
        o = results[c]["out"]                    # [4, 128] fp32
        cm = np.stack([o[0], o[1]], axis=1).ravel()   # j = 2p+u
        colmax = np.maximum(colmax, cm)
        diag[c * BL:(c + 1) * BL] = o[2, :BL]
        rowcosts[c * BL:(c + 1) * BL] = o[3, :BL]
    cost_im = np.maximum(np.float32(MARGIN) + colmax - diag, np.float32(0.0))
    loss = rowcosts.sum(dtype=np.float32) + cost_im.sum(dtype=np.float32)
    return np.array(loss, dtype=np.float32)


def _run(im, s, **spmd_kwargs):
    from concourse.bass_utils import run_bass_kernel_spmd

    im = np.ascontiguousarray(np.asarray(im), dtype=np.float32)
    s = np.ascontiguousarray(np.asarray(s), dtype=np.float32)
    nc = _get_nc()
    res = run_bass_kernel_spmd(nc, _make_in_maps(im, s), list(range(M)),
                               **spmd_kwargs)
    return _combine(res.results), res


def kernel(im, s):
    loss, _ = _run(im, s)
    return loss
